# revision 8
# baseline (speedup 1.0000x reference)
# Trainium2 Bass kernel for nn_BertAdapter_SLT_49933289783411
#
# Reference computation:
#   y   = tt_linear(x) + bias          (TT-factorized 768->768 linear)
#   out = x + gelu_exact(y)
#
# Key math: the TT cores with ranks [1,5,5,5,5,5,1] factor the 768x768
# weight as W = A @ B with A:(768,5), B:(5,768).  We precompute A,B on
# host (tiny, exact) and run a rank-5 bottleneck matmul on device.
#
# Sharding: data-parallel over the batch dim (8 batch elements -> 8 cores).
# Each core handles x_c:(512,768).  Host pre-transposes x_c to x^T (f-major)
# so the contraction dim lands on SBUF partitions; the device computes
#   t3    = A^T @ x^T              (5,512)   PSUM accumulate over 6 f-chunks
#   y^T_j = B_j^T @ t3             (128,512) per 128-feature output chunk j
#   o^T_j = x^T_j + gelu(y^T_j + bias_j)
# and the host transposes the gathered o^T back.
#
# The matmul path runs in bf16 (x cast on device, A/B cast on host): the
# TT branch contributes only ~4% of output magnitude, so bf16 there costs
# ~1e-4 relative output error while halving PE passes (fp32 matmul = 2
# passes on TRN2).  The residual add stays fully fp32.
#
# All constants (A, bias, B zero-padded to rank 128) are packed into the
# head of the single input tensor so one HWDGE queue carries everything —
# mixing SWDGE const loads with the x stream measurably delayed the x
# completion semaphores (SDMA engines round-robin between queues).

import numpy as np
import ml_dtypes

import concourse.bass as bass
import concourse.bacc as bacc
import concourse.mybir as mybir
import concourse.tile as tile
from concourse.bass_utils import run_bass_kernel_spmd

HID = 768
ROWS = 512          # rows per core (one batch element)
NCORES = 8
FCH = 6             # 768 / 128 feature chunks
RANK = 5
F32 = mybir.dt.float32
BF16 = mybir.dt.bfloat16

N_WARMUP = 20       # dummy PE matmuls to trip the HAM clock un-throttle

# packed layout of the input tensor, in f32 columns:
#   [A bf16 (128,30) -> 15] [bias f32 -> 6] [B_pad bf16 (128,768) -> 384] [x 6*512]
A_COLS = 15
BIAS_COLS = 6
BM_COLS = HID // 2
CONST_COLS = A_COLS + BIAS_COLS + BM_COLS          # 405
XT_COLS = CONST_COLS + FCH * ROWS                  # 3477

_CACHE = {}


def _build_program(act=None):
    if act is None:
        act = mybir.ActivationFunctionType.Gelu
    nc = bacc.Bacc(None, target_bir_lowering=False)
    xt = nc.dram_tensor("xt", [128, XT_COLS], F32, kind="ExternalInput")
    outt = nc.dram_tensor("outt", [128, FCH * ROWS], F32, kind="ExternalOutput")

    def xsl(c):
        return slice(CONST_COLS + c * ROWS, CONST_COLS + (c + 1) * ROWS)

    with tile.TileContext(nc) as tc:
        with (
            tc.tile_pool(name="const", bufs=1) as cpool,
            tc.tile_pool(name="xs", bufs=1) as xpool,
            tc.tile_pool(name="work", bufs=3) as wpool,
            tc.tile_pool(name="ps_t3", bufs=1, space="PSUM") as tpool,
            tc.tile_pool(name="ps_o", bufs=3, space="PSUM") as opool,
            tc.tile_pool(name="ps_w", bufs=1, space="PSUM") as wps_pool,
        ):
            # --- PE warmup: garbage matmuls so the HAM clock gate opens
            wsb = cpool.tile([128, 128], BF16)
            nc.gpsimd.memset(wsb[:], 0.0)
            wps = wps_pool.tile([128, 128], F32)
            for _ in range(N_WARMUP):
                nc.tensor.matmul(wps[:], wsb[:], wsb[:], start=True, stop=True)

            # t3 in bf16, zero-padded to 128 partitions so mm2 runs K=128
            t3_sb = cpool.tile([128, ROWS], BF16)
            nc.gpsimd.memset(t3_sb[:], 0.0)

            x_sb = xpool.tile([128, XT_COLS], F32)
            xb_sb = xpool.tile([128, FCH * ROWS], BF16)
            a_view = x_sb[:, 0:A_COLS].bitcast(BF16)                     # (128, 30)
            bias_view = x_sb[:, A_COLS : A_COLS + BIAS_COLS]             # (128, 6)
            bm_view = x_sb[:, A_COLS + BIAS_COLS : CONST_COLS].bitcast(BF16)  # (128, 768)

            t3_ps = tpool.tile([RANK, ROWS], F32)
            for c in range(FCH):
                if c == 0:
                    # consts + first x chunk in one transfer
                    nc.sync.dma_start(
                        x_sb[:, 0 : CONST_COLS + ROWS], xt[:, 0 : CONST_COLS + ROWS]
                    )
                else:
                    nc.sync.dma_start(x_sb[:, xsl(c)], xt[:, xsl(c)])
                bsl = slice(c * ROWS, (c + 1) * ROWS)
                nc.vector.tensor_copy(xb_sb[:, bsl], x_sb[:, xsl(c)])
                nc.tensor.matmul(
                    t3_ps[:],
                    a_view[:, c * RANK : (c + 1) * RANK],
                    xb_sb[:, bsl],
                    start=(c == 0),
                    stop=(c == FCH - 1),
                )

            nc.vector.tensor_copy(t3_sb[0:RANK, :], t3_ps[:])

            # --- per output chunk: y^T_j = B_j^T @ t3 ; o = x + gelu(y + b)
            for j in range(FCH):
                o_ps = opool.tile([128, ROWS], F32)
                nc.tensor.matmul(
                    o_ps[:],
                    bm_view[:, j * 128 : (j + 1) * 128],
                    t3_sb[:],
                    start=True,
                    stop=True,
                )
                # exact gelu with the TT bias folded in, in place in PSUM
                nc.scalar.activation(
                    o_ps[:],
                    o_ps[:],
                    act,
                    bias=bias_view[:, j : j + 1],
                    scale=1.0,
                )
                o_sb = wpool.tile([128, ROWS], F32)
                nc.vector.tensor_add(o_sb[:], o_ps[:], x_sb[:, xsl(j)])
                nc.sync.dma_start(outt[:, j * ROWS : (j + 1) * ROWS], o_sb[:])

    nc.finalize()
    return nc


def _get_program():
    if "nc" not in _CACHE:
        _CACHE["nc"] = _build_program()
    return _CACHE["nc"]


def _host_prep(hidden_states, bias, cores):
    """Collapse TT cores to rank-5 factors; pack consts + x^T per core."""
    c0, c1, c2, c3, c4, c5 = [c.astype(np.float64) for c in cores]
    A = np.einsum("iv,vjw,wkx->ijkx", c0[0], c1, c2).reshape(HID, RANK)
    Bm = np.einsum("xpy,yqz,zr->xpqr", c3, c4, c5[:, :, 0]).reshape(RANK, HID)

    a_p = np.ascontiguousarray(
        A.reshape(FCH, 128, RANK).transpose(1, 0, 2).reshape(128, FCH * RANK)
    ).astype(ml_dtypes.bfloat16)                    # (128, 30)
    bm_pad = np.zeros((128, HID), dtype=ml_dtypes.bfloat16)
    bm_pad[:RANK] = Bm.astype(ml_dtypes.bfloat16)  # (128, 768)
    bias_p = np.ascontiguousarray(bias.astype(np.float32).reshape(FCH, 128).T)

    const_block = np.concatenate(
        [
            a_p.view(np.float32),                  # (128, 15)
            bias_p,                                # (128, 6)
            bm_pad.view(np.float32),               # (128, 384)
        ],
        axis=1,
    ).astype(np.float32)                           # (128, 405)

    xts = []
    for c in range(NCORES):
        xc = hidden_states[c]  # (512, 768)
        xtc = np.ascontiguousarray(
            xc.T.reshape(FCH, 128, ROWS).transpose(1, 0, 2).reshape(128, FCH * ROWS)
        ).astype(np.float32)
        xts.append(np.ascontiguousarray(np.concatenate([const_block, xtc], axis=1)))
    return xts


def _unpack_out(outt_list):
    """outt[p, j*ROWS + m] = out[m, j*128 + p] -> (8, 512, 768)."""
    outs = []
    for outt in outt_list:
        o = outt.reshape(128, FCH, ROWS).transpose(2, 1, 0).reshape(ROWS, HID)
        outs.append(o)
    return np.stack(outs, axis=0).astype(np.float32)


def run(inputs, trace=False, **spmd_kwargs):
    hidden_states = np.asarray(inputs["hidden_states"], dtype=np.float32)
    bias = np.asarray(inputs["bias"], dtype=np.float32)
    cores = [np.asarray(inputs[f"core{i}"], dtype=np.float32) for i in range(6)]

    xts = _host_prep(hidden_states, bias, cores)
    nc = _get_program()
    in_maps = [{"xt": xts[c]} for c in range(NCORES)]
    res = run_bass_kernel_spmd(
        nc, in_maps, core_ids=list(range(NCORES)), trace=trace, **spmd_kwargs
    )
    out = _unpack_out([res.results[c]["outt"] for c in range(NCORES)])
    if trace:
        return out, res
    return out


def kernel(**inputs):
    return run(inputs)
